# revision 7
# baseline (speedup 1.0000x reference)
"""Trainium2 Bass kernel v3 for nn_CelltypeScaleLayer (segment gather + scale + transpose).

Reference computation:
    z = x[idx.reshape(-1)] * repeat(weight, M)[:, None]   # (NJ, NCELL)
    out = z.T.reshape(-1)                                 # (NCELL * NJ,)

v3 strategy (vs v2 at ~87us):
  - Same core idea as v2: fp16 SWDGE dma_gather with transpose=True lands
    row j's 512 cells as dst[c%128, c//128, j] (output layout), DVE applies
    the per-j weight, bf16 output, host widens to f32. x is pre-scaled by
    2^10 on the host (fp16 subnormal dodge), weights carry 2^-10.
  - SWDGE ring fix: the descriptor carveout is 1024 descs per queue, so with
    one queue a 896-index gather chunk fills the ring and chunk k+1's
    descriptor generation stalls until chunk k's transfer drains -> the
    gather leg serializes (~54us instead of 35.7us). v3 round-robins chunks
    across 4 SWDGE queues so gen/transfer pipeline across rings.
  - Output writes merged: one 3D dma_start per chunk (out DRAM laid out
    [128, 4, JPC] to match the SBUF tile) instead of 4 calls -> 14 instead
    of 56 HWDGE sequencer slots (565ns each) per iteration.
  - wrow (per-j weight row, 3.2MB DRAM read in v2) is gone: celltype
    boundaries are 500-aligned in every core's local j coordinates, so the
    host sends a [128, 26] per-500-block weight table and the device
    expands it once into a [128, NIDX] fp16 row via tensor_scalar
    broadcasts from a ones tile.

Per-core per-iteration DMA-engine traffic: 12.8MB gather read + 12.8MB
output write = 25.6MB @ ~360GB/s aggregate -> ~71us roofline.

Sharding: data-parallel over the NJ (gathered-row) axis; core m owns output
columns [m*JPC, (m+1)*JPC) of the (NCELL, NJ) transposed output, i.e. a
contiguous slab of the flattened output.
"""

import numpy as np
import ml_dtypes

import concourse.bacc as bacc
import concourse.tile as tile
import concourse.mybir as mybir
from concourse.bass_utils import run_bass_kernel_spmd

F32 = mybir.dt.float32
F16 = mybir.dt.float16
BF16 = mybir.dt.bfloat16
I16 = mybir.dt.int16

# Problem shape (hardcoded per the harness contract).
NF = 20000        # x rows (features)
NCELL = 512       # x cols (cells) == output rows
NCT = 50          # celltypes
M = 2000          # rows gathered per celltype
NJ = NCT * M      # 100000 gathered rows == output cols

NCORES = 8
JPC = NJ // NCORES          # 12500 output columns per core
# Gather chunks: the SWDGE descriptor ring holds dynamic_dma_scratch_size/16
# = 1024 descriptors per queue; one chunk must fit (1024 crashes, 896 fits)
# and num_idxs % 128 == 0 in transpose mode.
CHUNKS = [896] * 14
NIDX = sum(CHUNKS)          # 12544 (tail padded with index 0, never DMA'd out)
NQ = NCELL // 128           # 4 cell groups of 128
NQUEUES = 4                 # SWDGE queues (ucode max 4)
WBLK = 500                  # local-j weight block; ct boundaries are 500-aligned
NWB = NIDX // WBLK + 1      # 26: 25 full blocks + 44-col pad block

XSCALE = 1024.0             # host multiplies x by this, weight table divides it out

_cached = None


def _build(repeats=1, ncores=NCORES):
    nc = bacc.Bacc("TRN2", target_bir_lowering=False, debug=False,
                   num_devices=ncores, num_swdge_queues=NQUEUES)
    xh = nc.dram_tensor("xh", [NF, NCELL], F16, kind="ExternalInput")
    idxs = nc.dram_tensor("idxs", [128, NIDX // 16], I16, kind="ExternalInput")
    wcols = nc.dram_tensor("wcols", [128, NWB], F32, kind="ExternalInput")
    out = nc.dram_tensor("out", [128, NQ, JPC], F16, kind="ExternalOutput")

    # Per-chunk DMA-completion semaphores: Tile's rotating DMASW lane sems
    # assume in-order completion, which 4 concurrent SWDGE queues violate
    # (a laggard DMA engine's data can land after the lane's cumulative
    # target is reached). One sem per chunk slot, exact 16-inc-per-gather
    # targets, reused across repeats with growing targets.
    gsems = [nc.alloc_semaphore(f"gsem{i}") for i in range(len(CHUNKS))]

    with tile.TileContext(nc) as tc:
        with tc.tile_pool(name="const", bufs=1) as cpool:
            idx_sb = cpool.tile([128, NIDX // 16], I16)
            nc.sync.dma_start(idx_sb[:], idxs.ap())
            wc_sb = cpool.tile([128, NWB], F32)
            nc.sync.dma_start(wc_sb[:], wcols.ap())
            ones = cpool.tile([128, WBLK], F16)
            nc.vector.memset(ones[:], 1.0)
            # Expand the per-block weight table into a per-j row once.
            w_sb = cpool.tile([128, NIDX], F16)
            for s in range(NWB):
                n = min(WBLK, NIDX - s * WBLK)
                nc.vector.tensor_scalar_mul(
                    w_sb[:, s * WBLK:s * WBLK + n], ones[:, :n],
                    wc_sb[:, s:s + 1])

            with (
                tc.tile_pool(name="gpool", bufs=6) as gpool,
                tc.tile_pool(name="opool", bufs=6) as opool,
            ):
                ck_i = 0
                for rep in range(repeats):
                    j0 = 0
                    for ci, ck in enumerate(CHUNKS):
                        nvalid = max(0, min(JPC - j0, ck))
                        gb = gpool.tile([128, NQ, ck], F16, tag="gb")
                        nc.gpsimd.dma_gather(
                            gb[:],
                            xh.ap(),
                            idx_sb[:, j0 // 16:(j0 + ck) // 16],
                            ck,
                            ck,
                            NCELL,
                            transpose=True,
                            queue_num=ci % NQUEUES,
                        ).then_inc(gsems[ci], 16)
                        ob = opool.tile([128, NQ, ck], F16, tag="ob")
                        nc.vector.wait_ge(gsems[ci], 16 * (rep + 1))
                        for g in range(NQ):
                            nc.vector.tensor_tensor(
                                ob[:, g, :], gb[:, g, :],
                                w_sb[:, j0:j0 + ck],
                                op=mybir.AluOpType.mult,
                            )
                        nc.sync.dma_start(
                            out.ap()[:, :, j0:j0 + nvalid],
                            ob[:, :, :nvalid],
                        )
                        j0 += ck
                        ck_i += 1
    nc.compile()
    return nc


def _host_prep(x, weight, idx, ncores=NCORES):
    x32 = np.asarray(x, dtype=np.float32)
    xh = np.ascontiguousarray((x32 * XSCALE).astype(np.float16))
    weight = np.asarray(weight, dtype=np.float32)
    idx_flat = np.asarray(idx).reshape(-1).astype(np.int64)

    in_maps = []
    for m in range(ncores):
        j0 = m * JPC
        padded = np.zeros((NIDX,), dtype=np.int64)
        padded[:JPC] = idx_flat[j0:j0 + JPC]
        # dma_gather index layout: index i lives at partition i%16, free i//16,
        # replicated across the 8 Q7 core groups.
        wrapped16 = padded.reshape(NIDX // 16, 16).T.astype(np.int16)
        wrapped = np.ascontiguousarray(np.tile(wrapped16, (8, 1)))  # (128, NIDX//16)

        # Per-500-block weight table: local block s covers local j
        # [s*500, (s+1)*500), entirely within celltype (m*JPC + s*500)//M.
        wb = np.empty((NWB,), dtype=np.float32)
        for s in range(NWB):
            jloc = min(s * WBLK, JPC - 1)
            wb[s] = weight[(j0 + jloc) // M]
        wc = np.ascontiguousarray(
            np.broadcast_to(wb[None, :], (128, NWB)))

        in_maps.append({"xh": xh, "idxs": wrapped, "wcols": wc})
    return in_maps


def _run(inputs):
    global _cached
    if _cached is None:
        _cached = _build()
    nc = _cached
    in_maps = _host_prep(inputs["x"], inputs["weight"], inputs["idx"])
    res = run_bass_kernel_spmd(nc, in_maps, list(range(NCORES)))
    parts = []
    for m in range(NCORES):
        p = np.asarray(res.results[m]["out"])  # (128, NQ, JPC) f16 (x XSCALE)
        parts.append(p.transpose(1, 0, 2).reshape(NCELL, JPC))
    full = np.concatenate(parts, axis=1)  # (NCELL, NJ) f16, scaled by XSCALE
    return (np.ascontiguousarray(full).astype(np.float32)
            * (1.0 / XSCALE)).reshape(-1), res


def kernel(**inputs) -> np.ndarray:
    out, _ = _run(inputs)
    return out
